# revision 9
# baseline (speedup 1.0000x reference)
"""AffinityEnergyLoss on 8 Trainium2 NeuronCores.

Sharding: core k handles (layer l = k // 4, batch b = k % 4) — one
(l, b) slab of the encoder attns (8 heads x 1025 x 1025, CLS cropped)
plus the matching slab of decoder attns (8 heads x 1024 x 1024).

Per core, for each of its 16 maps M (1024 x 1024, fp32):
    rowsum s = M @ 1          (DVE reduce / ACT activation accum_out)
    r = 1/s                   (DVE reciprocal)
    S += diag(r) @ M          (PE fp32r matmul, accumulated in PSUM
                               over all 16 maps; diag(r) built as eye*r)
so S = sum_m D_m M_m for the core's maps. Then per 128-row block:
    T = S^T                   (PE transpose via identity)
    Z = S @ [softmax(preds_b) | 1]   (exact fp32 PE matmul over T chunks)
Z (1024 x 22) is the core's partial of sum_m D_m M_m @ [P|1].

Host: affinity_raw_b = (Z_{l=0,b} + Z_{l=1,b}) / 32, row-normalize the
first 21 columns, then loss = sum(roi * |prob - affinity|) / N.
"""
import numpy as np

import concourse.bacc as bacc
import concourse.mybir as mybir
import concourse.tile as tile
from concourse.bass_utils import run_bass_kernel_spmd

F32 = mybir.dt.float32
F32R = mybir.dt.float32r
AX = mybir.AxisListType.X
ACTF = mybir.ActivationFunctionType

HEADS = 8
TOK = 1024
C = 21
PB = 128          # partition block
NBLK = TOK // PB  # 8

_NC = None


def _build_nc():
    nc = bacc.Bacc(None, target_bir_lowering=False)
    enc = nc.dram_tensor("enc", [HEADS, 1025, 1025], F32, kind="ExternalInput")
    dec = nc.dram_tensor("dec", [HEADS, TOK, TOK], F32, kind="ExternalInput")
    pt = nc.dram_tensor("pt", [TOK, C], F32, kind="ExternalInput")
    eye = nc.dram_tensor("eye", [PB, PB], F32, kind="ExternalInput")
    z = nc.dram_tensor("z", [NBLK, C, PB], F32, kind="ExternalOutput")

    HCHUNK = 2   # heads per DMA chunk; with row-pairing each chunk is 2 MB
    NPAIR = 4    # block-pairs of 256 map rows
    ENCW = 2050  # two full 1025-wide enc rows per partition (CLS col sliced off)
    DECW = 2048

    def _chunk_src(enc, dec, ip, c4):
        # rows 2p+q of the 256-row group land in partition p, contiguous
        r0 = ip * 256
        if c4 < 4:
            h0 = c4 * HCHUNK
            return enc[h0 : h0 + HCHUNK, 1 + r0 : 1 + r0 + 256, :].rearrange(
                "h (p q) j -> p h (q j)", q=2
            )
        h0 = (c4 - 4) * HCHUNK
        return dec[h0 : h0 + HCHUNK, r0 : r0 + 256, :].rearrange(
            "h (p q) j -> p h (q j)", q=2
        )

    def _plane(t, c4, hm, q):
        # [128, 1024] view of one parity plane of one map
        if c4 < 4:
            return t[:, hm, q * 1025 + 1 : q * 1025 + 1025]
        return t[:, hm, q * 1024 : (q + 1) * 1024]

    with tile.TileContext(nc) as tc:
        with (
            tc.tile_pool(name="const", bufs=1) as const,
            tc.tile_pool(name="stats", bufs=8) as stats,
            tc.tile_pool(name="big", bufs=6) as big,
            tc.tile_pool(name="spool", bufs=2) as spool,
            tc.tile_pool(name="zout", bufs=2) as zout,
            tc.tile_pool(name="psS", bufs=1, space="PSUM") as psS,
            tc.tile_pool(name="psT", bufs=2, space="PSUM") as psT,
            tc.tile_pool(name="psZ", bufs=2, space="PSUM") as psZ,
        ):
            # issue the first block-pair's loads before anything else
            chunk_tiles = {}
            for c4 in range(8):
                t = big.tile([PB, HCHUNK, ENCW], F32R, tag="chunk")
                nc.gpsimd.dma_start(
                    out=t[:, :, 0 : ENCW if c4 < 4 else DECW],
                    in_=_chunk_src(enc, dec, 0, c4),
                )
                chunk_tiles[(0, c4)] = t

            eye_sb = const.tile([PB, PB], F32)
            nc.sync.dma_start(out=eye_sb[:], in_=eye[:])

            pt_sb = const.tile([PB, NBLK, C], F32)
            nc.sync.dma_start(
                out=pt_sb[:], in_=pt.rearrange("(c p) n -> p c n", p=PB)
            )
            pa_sb = const.tile([PB, NBLK, C], F32R)
            for c in range(NBLK):
                negmx = stats.tile([PB, 1], F32, tag="negmx")
                nc.vector.reduce_max(negmx[:], pt_sb[:, c, :], axis=AX, negate=True)
                ssum = stats.tile([PB, 1], F32, tag="ssum")
                ex = stats.tile([PB, C], F32, tag="ex")
                nc.scalar.activation(
                    ex[:],
                    pt_sb[:, c, :],
                    ACTF.Exp,
                    bias=negmx[:],
                    accum_out=ssum[:],
                )
                rs = stats.tile([PB, 1], F32, tag="rs")
                nc.vector.reciprocal(rs[:], ssum[:])
                nc.vector.tensor_scalar_mul(pa_sb[:, c, :], ex[:], rs[:])

            for ip in range(NPAIR):
                S_ps = [
                    psS.tile([PB, TOK], F32, tag=f"S{q}", name=f"S_ps{q}")
                    for q in range(2)
                ]
                for c4 in range(8):
                    t = chunk_tiles.pop((ip, c4), None)
                    if t is None:
                        t = big.tile([PB, HCHUNK, ENCW], F32R, tag="chunk")
                        nc.gpsimd.dma_start(
                            out=t[:, :, 0 : ENCW if c4 < 4 else DECW],
                            in_=_chunk_src(enc, dec, ip, c4),
                        )
                    for hm in range(HCHUNK):
                        m = (c4 % 4) * HCHUNK + hm + (0 if c4 < 4 else 8)
                        for q in range(2):
                            src = _plane(t, c4, hm, q)
                            s_m = stats.tile([PB, 1], F32, tag="s_m")
                            if (2 * m + q) % 2 == 0:
                                nc.vector.reduce_sum(
                                    s_m[:], src.bitcast(F32), axis=AX
                                )
                            else:
                                scr = spool.tile([PB, TOK], F32, tag="scr")
                                nc.scalar.activation(
                                    scr[:],
                                    src.bitcast(F32),
                                    ACTF.Copy,
                                    accum_out=s_m[:],
                                )
                            r_m = stats.tile([PB, 1], F32, tag="r_m")
                            nc.vector.reciprocal(r_m[:], s_m[:])
                            dg = stats.tile([PB, PB], F32R, tag="dg")
                            nc.vector.tensor_scalar_mul(dg[:], eye_sb[:], r_m[:])
                            first = c4 == 0 and hm == 0
                            last = c4 == 7 and hm == HCHUNK - 1
                            nc.tensor.matmul(
                                S_ps[q][:, 0:512], dg[:], src[:, 0:512],
                                start=first, stop=last,
                            )
                            nc.tensor.matmul(
                                S_ps[q][:, 512:1024], dg[:], src[:, 512:1024],
                                start=first, stop=last,
                            )

                for q in range(2):
                    S_sb = spool.tile([PB, TOK], F32, tag="S")
                    nc.scalar.copy(out=S_sb[:], in_=S_ps[q][:])
                    T_sb = spool.tile([PB, NBLK, PB], F32R, tag="T")
                    for jc in range(NBLK):
                        tp = psT.tile([PB, PB], F32)
                        nc.tensor.transpose(
                            tp[:], S_sb[:, jc * PB : (jc + 1) * PB], eye_sb[:]
                        )
                        nc.scalar.copy(out=T_sb[:, jc, :], in_=tp[:])
                    z_ps = psZ.tile([C, PB], F32)
                    for jc in range(NBLK):
                        nc.tensor.matmul(
                            z_ps[:], pa_sb[:, jc, :], T_sb[:, jc, :],
                            start=(jc == 0), stop=(jc == NBLK - 1),
                        )
                    z_sb = zout.tile([C, PB], F32)
                    nc.vector.tensor_copy(z_sb[:], z_ps[:])
                    nc.sync.dma_start(out=z[2 * ip + q, :, :], in_=z_sb[:])

    nc.compile()
    return nc


def _get_nc():
    global _NC
    if _NC is None:
        _NC = _build_nc()
    return _NC


def kernel(preds, low_feats, high_feats, unlabeled_ROIs, targets, attns, decode_attns):
    preds = np.asarray(preds, dtype=np.float32)
    attns = np.asarray(attns, dtype=np.float32)
    decode_attns = np.asarray(decode_attns, dtype=np.float32)
    roi = np.asarray(unlabeled_ROIs)

    bz = preds.shape[0]
    preds_t = np.ascontiguousarray(
        preds.reshape(bz, C, TOK).transpose(0, 2, 1)
    )  # (bz, 1024, 21)
    eye_np = np.eye(PB, dtype=np.float32)

    nc = _get_nc()
    in_maps = []
    for k in range(8):
        l, b = k // 4, k % 4
        in_maps.append(
            {
                "enc": np.ascontiguousarray(attns[l, b]),
                "dec": np.ascontiguousarray(decode_attns[l, b]),
                "pt": preds_t[b],
                "eye": eye_np,
            }
        )
    res = run_bass_kernel_spmd(nc, in_maps, core_ids=list(range(8)))
    # z per core: (2*ip+q, C, p) with Z[ip*256 + 2p + q, c] = z[2ip+q, c, p]
    def _dec(zarr):
        a = zarr.reshape(4, 2, C, PB)          # (ip, q, c, p)
        a = a.transpose(0, 3, 1, 2)            # (ip, p, q, c)
        return a.reshape(TOK, C)
    zs = np.stack([_dec(res.results[k]["z"]) for k in range(8)])

    # combine: affinity_raw_b = (Z_{l=0,b} + Z_{l=1,b}) / 32
    zb = zs.reshape(2, bz, TOK, C).sum(axis=0) / 32.0
    aff = zb / zb.sum(axis=-1, keepdims=True)

    # host softmax (matches jax.nn.softmax in f32)
    e = np.exp(preds_t - preds_t.max(axis=-1, keepdims=True))
    prob = e / e.sum(axis=-1, keepdims=True)  # (bz, 1024, 21)

    roi_f = roi.astype(np.float32).reshape(bz, TOK, 1)
    n_roi = roi_f.sum()
    loss = (roi_f * np.abs(prob - aff)).sum()
    if n_roi > 0:
        loss = loss / n_roi
    return np.asarray(loss, dtype=np.float32)


# revision 12
# speedup vs baseline: 1.0313x; 1.0313x over previous
"""AffinityEnergyLoss on 8 Trainium2 NeuronCores.

Sharding: core k handles (layer l = k // 4, batch b = k % 4) — one
(l, b) slab of the encoder attns (8 heads x 1025 x 1025, CLS cropped)
plus the matching slab of decoder attns (8 heads x 1024 x 1024).

Per core, for each of its 16 maps M (1024 x 1024, fp32):
    rowsum s = M @ 1          (DVE reduce / ACT activation accum_out)
    r = 1/s                   (DVE reciprocal)
    S += diag(r) @ M          (PE fp32r matmul, accumulated in PSUM
                               over all 16 maps; diag(r) built as eye*r)
so S = sum_m D_m M_m for the core's maps. Then per 128-row block:
    T = S^T                   (PE transpose via identity)
    Z = S @ [softmax(preds_b) | 1]   (exact fp32 PE matmul over T chunks)
Z (1024 x 22) is the core's partial of sum_m D_m M_m @ [P|1].

Host: affinity_raw_b = (Z_{l=0,b} + Z_{l=1,b}) / 32, row-normalize the
first 21 columns, then loss = sum(roi * |prob - affinity|) / N.
"""
import numpy as np

import concourse.bacc as bacc
import concourse.mybir as mybir
import concourse.tile as tile
from concourse.bass_utils import run_bass_kernel_spmd

F32 = mybir.dt.float32
F32R = mybir.dt.float32r
AX = mybir.AxisListType.X
ACTF = mybir.ActivationFunctionType

HEADS = 8
TOK = 1024
C = 21
PB = 128          # partition block
NBLK = TOK // PB  # 8

_NC = None


def _build_nc():
    nc = bacc.Bacc(None, target_bir_lowering=False)
    enc = nc.dram_tensor("enc", [HEADS, 1025, 1025], F32, kind="ExternalInput")
    dec = nc.dram_tensor("dec", [HEADS, TOK, TOK], F32, kind="ExternalInput")
    pt = nc.dram_tensor("pt", [TOK, C], F32, kind="ExternalInput")
    eye = nc.dram_tensor("eye", [PB, PB], F32, kind="ExternalInput")
    z = nc.dram_tensor("z", [NBLK, C, PB], F32, kind="ExternalOutput")

    HCHUNK = 2   # heads per DMA chunk; with row-pairing each chunk is 2 MB
    NPAIR = 4    # block-pairs of 256 map rows
    ENCW = 2048
    DECW = 2048

    def _load_chunk(nc, t, enc, dec, ip, c4):
        # rows 2p+q of the 256-row group land in partition p, plane q
        r0 = ip * 256
        if c4 < 4:
            h0 = c4 * HCHUNK
            for qq in range(2):
                nc.gpsimd.dma_start(
                    out=t[:, :, qq, :],
                    in_=enc[
                        h0 : h0 + HCHUNK, 1 + r0 + qq : 1 + r0 + 256 : 2, 1:
                    ].transpose([1, 0, 2]),
                )
        else:
            h0 = (c4 - 4) * HCHUNK
            nc.gpsimd.dma_start(
                out=t[:],
                in_=dec[h0 : h0 + HCHUNK, r0 : r0 + 256, :].rearrange(
                    "h (p q) j -> p h (q j)", q=2
                ),
            )

    def _plane(t, c4, hm, q):
        # [128, 1024] view of one parity plane of one map
        return t[:, hm, q, :]

    with tile.TileContext(nc) as tc:
        with (
            tc.tile_pool(name="const", bufs=1) as const,
            tc.tile_pool(name="stats", bufs=8) as stats,
            tc.tile_pool(name="big", bufs=6) as big,
            tc.tile_pool(name="spool", bufs=2) as spool,
            tc.tile_pool(name="zout", bufs=2) as zout,
            tc.tile_pool(name="psS", bufs=1, space="PSUM") as psS,
            tc.tile_pool(name="psT", bufs=2, space="PSUM") as psT,
            tc.tile_pool(name="psZ", bufs=2, space="PSUM") as psZ,
        ):
            # issue the first block-pair's loads before anything else
            chunk_tiles = {}
            for c4 in range(8):
                t = big.tile([PB, HCHUNK, 2, TOK], F32R, tag="chunk")
                _load_chunk(nc, t, enc, dec, 0, c4)
                chunk_tiles[(0, c4)] = t

            eye_sb = const.tile([PB, PB], F32)
            nc.sync.dma_start(out=eye_sb[:], in_=eye[:])

            pt_sb = const.tile([PB, NBLK, C], F32)
            nc.sync.dma_start(
                out=pt_sb[:], in_=pt.rearrange("(c p) n -> p c n", p=PB)
            )
            pa_sb = const.tile([PB, NBLK, C], F32R)
            for c in range(NBLK):
                negmx = stats.tile([PB, 1], F32, tag="negmx")
                nc.vector.reduce_max(negmx[:], pt_sb[:, c, :], axis=AX, negate=True)
                ssum = stats.tile([PB, 1], F32, tag="ssum")
                ex = stats.tile([PB, C], F32, tag="ex")
                nc.scalar.activation(
                    ex[:],
                    pt_sb[:, c, :],
                    ACTF.Exp,
                    bias=negmx[:],
                    accum_out=ssum[:],
                )
                rs = stats.tile([PB, 1], F32, tag="rs")
                nc.vector.reciprocal(rs[:], ssum[:])
                nc.vector.tensor_scalar_mul(pa_sb[:, c, :], ex[:], rs[:])

            for ip in range(NPAIR):
                S_ps = [
                    psS.tile([PB, TOK], F32, tag=f"S{q}", name=f"S_ps{q}")
                    for q in range(2)
                ]
                for c4 in range(8):
                    t = chunk_tiles.pop((ip, c4), None)
                    if t is None:
                        t = big.tile([PB, HCHUNK, 2, TOK], F32R, tag="chunk")
                        _load_chunk(nc, t, enc, dec, ip, c4)
                    for hm in range(HCHUNK):
                        m = (c4 % 4) * HCHUNK + hm + (0 if c4 < 4 else 8)
                        for q in range(2):
                            src = _plane(t, c4, hm, q)
                            s_m = stats.tile([PB, 1], F32, tag="s_m")
                            if (2 * m + q) % 2 == 0:
                                nc.vector.reduce_sum(
                                    s_m[:], src.bitcast(F32), axis=AX
                                )
                            else:
                                scr = spool.tile([PB, TOK], F32, tag="scr")
                                nc.scalar.activation(
                                    scr[:],
                                    src.bitcast(F32),
                                    ACTF.Copy,
                                    accum_out=s_m[:],
                                )
                            r_m = stats.tile([PB, 1], F32, tag="r_m")
                            nc.vector.reciprocal(r_m[:], s_m[:])
                            dg = stats.tile([PB, PB], F32R, tag="dg")
                            nc.vector.tensor_scalar_mul(dg[:], eye_sb[:], r_m[:])
                            first = c4 == 0 and hm == 0
                            last = c4 == 7 and hm == HCHUNK - 1
                            nc.tensor.matmul(
                                S_ps[q][:, 0:512], dg[:], src[:, 0:512],
                                start=first, stop=last,
                            )
                            nc.tensor.matmul(
                                S_ps[q][:, 512:1024], dg[:], src[:, 512:1024],
                                start=first, stop=last,
                            )

                for q in range(2):
                    S_sb = spool.tile([PB, TOK], F32, tag="S")
                    nc.scalar.copy(out=S_sb[:], in_=S_ps[q][:])
                    T_sb = spool.tile([PB, NBLK, PB], F32R, tag="T")
                    for jc in range(NBLK):
                        tp = psT.tile([PB, PB], F32)
                        nc.tensor.transpose(
                            tp[:], S_sb[:, jc * PB : (jc + 1) * PB], eye_sb[:]
                        )
                        nc.scalar.copy(out=T_sb[:, jc, :], in_=tp[:])
                    z_ps = psZ.tile([C, PB], F32)
                    for jc in range(NBLK):
                        nc.tensor.matmul(
                            z_ps[:], pa_sb[:, jc, :], T_sb[:, jc, :],
                            start=(jc == 0), stop=(jc == NBLK - 1),
                        )
                    z_sb = zout.tile([C, PB], F32)
                    nc.vector.tensor_copy(z_sb[:], z_ps[:])
                    nc.sync.dma_start(out=z[2 * ip + q, :, :], in_=z_sb[:])

    nc.compile()
    return nc


def _get_nc():
    global _NC
    if _NC is None:
        _NC = _build_nc()
    return _NC


def kernel(preds, low_feats, high_feats, unlabeled_ROIs, targets, attns, decode_attns):
    preds = np.asarray(preds, dtype=np.float32)
    attns = np.asarray(attns, dtype=np.float32)
    decode_attns = np.asarray(decode_attns, dtype=np.float32)
    roi = np.asarray(unlabeled_ROIs)

    bz = preds.shape[0]
    preds_t = np.ascontiguousarray(
        preds.reshape(bz, C, TOK).transpose(0, 2, 1)
    )  # (bz, 1024, 21)
    eye_np = np.eye(PB, dtype=np.float32)

    nc = _get_nc()
    in_maps = []
    for k in range(8):
        l, b = k // 4, k % 4
        in_maps.append(
            {
                "enc": np.ascontiguousarray(attns[l, b]),
                "dec": np.ascontiguousarray(decode_attns[l, b]),
                "pt": preds_t[b],
                "eye": eye_np,
            }
        )
    res = run_bass_kernel_spmd(nc, in_maps, core_ids=list(range(8)))
    # z per core: (2*ip+q, C, p) with Z[ip*256 + 2p + q, c] = z[2ip+q, c, p]
    def _dec(zarr):
        a = zarr.reshape(4, 2, C, PB)          # (ip, q, c, p)
        a = a.transpose(0, 3, 1, 2)            # (ip, p, q, c)
        return a.reshape(TOK, C)
    zs = np.stack([_dec(res.results[k]["z"]) for k in range(8)])

    # combine: affinity_raw_b = (Z_{l=0,b} + Z_{l=1,b}) / 32
    zb = zs.reshape(2, bz, TOK, C).sum(axis=0) / 32.0
    aff = zb / zb.sum(axis=-1, keepdims=True)

    # host softmax (matches jax.nn.softmax in f32)
    e = np.exp(preds_t - preds_t.max(axis=-1, keepdims=True))
    prob = e / e.sum(axis=-1, keepdims=True)  # (bz, 1024, 21)

    roi_f = roi.astype(np.float32).reshape(bz, TOK, 1)
    n_roi = roi_f.sum()
    loss = (roi_f * np.abs(prob - aff)).sum()
    if n_roi > 0:
        loss = loss / n_roi
    return np.asarray(loss, dtype=np.float32)


# revision 13
# speedup vs baseline: 1.1799x; 1.1441x over previous
"""AffinityEnergyLoss on 8 Trainium2 NeuronCores.

Sharding: core k handles (layer l = k // 4, batch b = k % 4) — one
(l, b) slab of the encoder attns (8 heads x 1025 x 1025, CLS cropped)
plus the matching slab of decoder attns (8 heads x 1024 x 1024).

Per core, for each of its 16 maps M (1024 x 1024, fp32):
    rowsum s = M @ 1          (DVE reduce / ACT activation accum_out)
    r = 1/s                   (DVE reciprocal)
    S += diag(r) @ M          (PE fp32r matmul, accumulated in PSUM
                               over all 16 maps; diag(r) built as eye*r)
so S = sum_m D_m M_m for the core's maps. Then per 128-row block:
    T = S^T                   (PE transpose via identity)
    Z^T = Pa^T @ T            (fp32r PE matmul, Pa = softmax(preds_b))
Z (1024 x 21) is the core's partial of sum_m D_m M_m @ P.

Host: affinity_raw_b = (Z_{l=0,b} + Z_{l=1,b}) / 32, row-normalize,
then loss = sum(roi * |prob - affinity|) / N.
"""
import numpy as np

import concourse.bacc as bacc
import concourse.mybir as mybir
import concourse.tile as tile
from concourse.bass_utils import run_bass_kernel_spmd

F32 = mybir.dt.float32
F32R = mybir.dt.float32r
AX = mybir.AxisListType.X
ACTF = mybir.ActivationFunctionType

HEADS = 8
TOK = 1024
C = 21
PB = 128          # partition block
NBLK = TOK // PB  # 8

_NC = None


def _build_nc():
    nc = bacc.Bacc(None, target_bir_lowering=False)
    enc = nc.dram_tensor("enc", [HEADS, 1025, 1025], F32, kind="ExternalInput")
    dec = nc.dram_tensor("dec", [HEADS, TOK, TOK], F32, kind="ExternalInput")
    pt = nc.dram_tensor("pt", [TOK, C], F32, kind="ExternalInput")
    eye = nc.dram_tensor("eye", [PB, PB], F32, kind="ExternalInput")
    z = nc.dram_tensor("z", [NBLK, C, PB], F32, kind="ExternalOutput")

    HCHUNK = 4  # heads per DMA chunk (2 MB loads)

    def _chunk_src(enc, dec, ib, c4):
        i0 = ib * PB
        if c4 < 2:
            h0 = c4 * HCHUNK
            return enc[h0 : h0 + HCHUNK, 1 + i0 : 1 + i0 + PB, 1:].transpose([1, 0, 2])
        h0 = (c4 - 2) * HCHUNK
        return dec[h0 : h0 + HCHUNK, i0 : i0 + PB, :].transpose([1, 0, 2])

    with tile.TileContext(nc) as tc:
        with (
            tc.tile_pool(name="const", bufs=1) as const,
            tc.tile_pool(name="stats", bufs=8) as stats,
            tc.tile_pool(name="big", bufs=6) as big,
            tc.tile_pool(name="spool", bufs=2) as spool,
            tc.tile_pool(name="zout", bufs=2) as zout,
            tc.tile_pool(name="psS", bufs=2, space="PSUM") as psS,
            tc.tile_pool(name="psT", bufs=2, space="PSUM") as psT,
            tc.tile_pool(name="psZ", bufs=2, space="PSUM") as psZ,
        ):
            # issue the first block's big loads before anything else
            chunk_tiles = {}
            for c4 in range(4):
                t = big.tile([PB, HCHUNK, TOK], F32R, tag="chunk")
                nc.gpsimd.dma_start(out=t[:], in_=_chunk_src(enc, dec, 0, c4))
                chunk_tiles[(0, c4)] = t

            eye_sb = const.tile([PB, PB], F32)
            nc.sync.dma_start(out=eye_sb[:], in_=eye[:])

            pt_sb = const.tile([PB, NBLK, C], F32)
            nc.sync.dma_start(
                out=pt_sb[:], in_=pt.rearrange("(c p) n -> p c n", p=PB)
            )
            pa_sb = const.tile([PB, NBLK, C], F32R)
            for c in range(NBLK):
                negmx = stats.tile([PB, 1], F32, tag="negmx")
                nc.vector.reduce_max(negmx[:], pt_sb[:, c, :], axis=AX, negate=True)
                ssum = stats.tile([PB, 1], F32, tag="ssum")
                ex = stats.tile([PB, C], F32, tag="ex")
                nc.scalar.activation(
                    ex[:],
                    pt_sb[:, c, :],
                    ACTF.Exp,
                    bias=negmx[:],
                    accum_out=ssum[:],
                )
                rs = stats.tile([PB, 1], F32, tag="rs")
                nc.vector.reciprocal(rs[:], ssum[:])
                nc.vector.tensor_scalar_mul(pa_sb[:, c, :], ex[:], rs[:])

            for ib in range(NBLK):
                S_ps = psS.tile([PB, TOK], F32)
                for c4 in range(4):
                    t = chunk_tiles.pop((ib, c4), None)
                    if t is None:
                        t = big.tile([PB, HCHUNK, TOK], F32R, tag="chunk")
                        nc.gpsimd.dma_start(out=t[:], in_=_chunk_src(enc, dec, ib, c4))
                    for hm in range(HCHUNK):
                        m = c4 * HCHUNK + hm
                        src = t[:, hm, :]
                        s_m = stats.tile([PB, 1], F32, tag="s_m")
                        if m % 2 == 0:
                            nc.vector.reduce_sum(s_m[:], src.bitcast(F32), axis=AX)
                        else:
                            scr = spool.tile([PB, TOK], F32, tag="scr")
                            nc.scalar.activation(
                                scr[:], src.bitcast(F32), ACTF.Copy, accum_out=s_m[:]
                            )
                        r_m = stats.tile([PB, 1], F32, tag="r_m")
                        nc.vector.reciprocal(r_m[:], s_m[:])
                        dg = stats.tile([PB, PB], F32R, tag="dg")
                        nc.vector.tensor_scalar_mul(dg[:], eye_sb[:], r_m[:])
                        nc.tensor.matmul(
                            S_ps[:, 0:512], dg[:], src[:, 0:512],
                            start=(m == 0), stop=(m == 15),
                        )
                        nc.tensor.matmul(
                            S_ps[:, 512:1024], dg[:], src[:, 512:1024],
                            start=(m == 0), stop=(m == 15),
                        )

                S_sb = spool.tile([PB, TOK], F32, tag="S")
                nc.scalar.copy(out=S_sb[:], in_=S_ps[:])
                T_sb = spool.tile([PB, NBLK, PB], F32R, tag="T")
                for jc in range(NBLK):
                    tp = psT.tile([PB, PB], F32)
                    nc.tensor.transpose(
                        tp[:], S_sb[:, jc * PB : (jc + 1) * PB], eye_sb[:]
                    )
                    nc.scalar.copy(out=T_sb[:, jc, :], in_=tp[:])
                z_ps = psZ.tile([C, PB], F32)
                for jc in range(NBLK):
                    nc.tensor.matmul(
                        z_ps[:], pa_sb[:, jc, :], T_sb[:, jc, :],
                        start=(jc == 0), stop=(jc == NBLK - 1),
                    )
                z_sb = zout.tile([C, PB], F32)
                nc.vector.tensor_copy(z_sb[:], z_ps[:])
                nc.sync.dma_start(out=z[ib, :, :], in_=z_sb[:])

    nc.compile()
    return nc


def _get_nc():
    global _NC
    if _NC is None:
        _NC = _build_nc()
    return _NC


def kernel(preds, low_feats, high_feats, unlabeled_ROIs, targets, attns, decode_attns):
    preds = np.asarray(preds, dtype=np.float32)
    attns = np.asarray(attns, dtype=np.float32)
    decode_attns = np.asarray(decode_attns, dtype=np.float32)
    roi = np.asarray(unlabeled_ROIs)

    bz = preds.shape[0]
    preds_t = np.ascontiguousarray(
        preds.reshape(bz, C, TOK).transpose(0, 2, 1)
    )  # (bz, 1024, 21)
    eye_np = np.eye(PB, dtype=np.float32)

    nc = _get_nc()
    in_maps = []
    for k in range(8):
        l, b = k // 4, k % 4
        in_maps.append(
            {
                "enc": np.ascontiguousarray(attns[l, b]),
                "dec": np.ascontiguousarray(decode_attns[l, b]),
                "pt": preds_t[b],
                "eye": eye_np,
            }
        )
    res = run_bass_kernel_spmd(nc, in_maps, core_ids=list(range(8)))
    # z per core: (NBLK, C, PB) holding Z^T per block -> (1024, 21)
    zs = np.stack(
        [
            res.results[k]["z"].transpose(0, 2, 1).reshape(TOK, C)
            for k in range(8)
        ]
    )

    # combine: affinity_raw_b = (Z_{l=0,b} + Z_{l=1,b}) / 32
    zb = zs.reshape(2, bz, TOK, C).sum(axis=0) / 32.0
    aff = zb / zb.sum(axis=-1, keepdims=True)

    # host softmax (matches jax.nn.softmax in f32)
    e = np.exp(preds_t - preds_t.max(axis=-1, keepdims=True))
    prob = e / e.sum(axis=-1, keepdims=True)  # (bz, 1024, 21)

    roi_f = roi.astype(np.float32).reshape(bz, TOK, 1)
    n_roi = roi_f.sum()
    loss = (roi_f * np.abs(prob - aff)).sum()
    if n_roi > 0:
        loss = loss / n_roi
    return np.asarray(loss, dtype=np.float32)


# revision 15
# speedup vs baseline: 1.2113x; 1.0267x over previous
"""AffinityEnergyLoss on 8 Trainium2 NeuronCores.

Sharding: core k handles (layer l = k // 4, batch b = k % 4) — one
(l, b) slab of the encoder attns (8 heads x 1025 x 1025, CLS cropped)
plus the matching slab of decoder attns (8 heads x 1024 x 1024).

Per core, for each of its 16 maps M (1024 x 1024, fp32):
    rowsum s = M @ 1          (DVE reduce / ACT activation accum_out)
    r = 1/s                   (DVE reciprocal)
    S += diag(r) @ M          (PE fp32r matmul, accumulated in PSUM
                               over all 16 maps; diag(r) built as eye*r)
so S = sum_m D_m M_m for the core's maps. Then per 128-row block:
    T = S^T                   (PE transpose via identity)
    Z^T = Pa^T @ T            (fp32r PE matmul, Pa = softmax(preds_b))
Z (1024 x 21) is the core's partial of sum_m D_m M_m @ P.

Host: affinity_raw_b = (Z_{l=0,b} + Z_{l=1,b}) / 32, row-normalize,
then loss = sum(roi * |prob - affinity|) / N.
"""
import numpy as np

import concourse.bacc as bacc
import concourse.mybir as mybir
import concourse.tile as tile
from concourse.bass_utils import run_bass_kernel_spmd

F32 = mybir.dt.float32
F32R = mybir.dt.float32r
AX = mybir.AxisListType.X
ACTF = mybir.ActivationFunctionType

HEADS = 8
TOK = 1024
C = 21
PB = 128          # partition block
NBLK = TOK // PB  # 8

_NC = None


def _build_nc():
    nc = bacc.Bacc(None, target_bir_lowering=False)
    enc = nc.dram_tensor("enc", [HEADS, 1025, 1025], F32, kind="ExternalInput")
    dec = nc.dram_tensor("dec", [HEADS, TOK, TOK], F32, kind="ExternalInput")
    pt = nc.dram_tensor("pt", [TOK, C], F32, kind="ExternalInput")
    eye = nc.dram_tensor("eye", [PB, PB], F32, kind="ExternalInput")
    z = nc.dram_tensor("z", [NBLK, C, PB], F32, kind="ExternalOutput")

    def _head_src(enc, dec, ib, m, nh):
        # heads m..m+nh of the block's 16 maps (0-7 enc, 8-15 dec)
        i0 = ib * PB
        if m < 8:
            return enc[m : m + nh, 1 + i0 : 1 + i0 + PB, 1:].transpose([1, 0, 2])
        return dec[m - 8 : m - 8 + nh, i0 : i0 + PB, :].transpose([1, 0, 2])

    def _chunk_plan(ib):
        # (start_map, n_heads) per DMA chunk for block ib
        if ib == 0:
            # small leading chunks so the first bytes land ASAP
            return [(0, 1), (1, 1), (2, 2), (4, 4), (8, 4), (12, 4)]
        if ib == NBLK - 1:
            # small trailing chunks to shrink the serial tail
            return [(0, 4), (4, 4), (8, 4), (12, 2), (14, 1), (15, 1)]
        return [(0, 4), (4, 4), (8, 4), (12, 4)]

    with tile.TileContext(nc) as tc:
        with (
            tc.tile_pool(name="const", bufs=1) as const,
            tc.tile_pool(name="stats", bufs=8) as stats,
            tc.tile_pool(name="big", bufs=6) as big,
            tc.tile_pool(name="spool", bufs=2) as spool,
            tc.tile_pool(name="zout", bufs=2) as zout,
            tc.tile_pool(name="psS", bufs=2, space="PSUM") as psS,
            tc.tile_pool(name="psT", bufs=2, space="PSUM") as psT,
            tc.tile_pool(name="psZ", bufs=2, space="PSUM") as psZ,
        ):
            # issue the first block's big loads before anything else
            chunk_tiles = {}
            for ci, (m0, nh) in enumerate(_chunk_plan(0)):
                t = big.tile([PB, 4, TOK], F32R, tag="chunk", name=f"pre{ci}")
                nc.gpsimd.dma_start(
                    out=t[:, 0:nh, :], in_=_head_src(enc, dec, 0, m0, nh)
                )
                chunk_tiles[(0, ci)] = t

            eye_sb = const.tile([PB, PB], F32)
            nc.sync.dma_start(out=eye_sb[:], in_=eye[:])

            pt_sb = const.tile([PB, NBLK, C], F32)
            nc.sync.dma_start(
                out=pt_sb[:], in_=pt.rearrange("(c p) n -> p c n", p=PB)
            )
            pa_sb = const.tile([PB, NBLK, C], F32R)
            for c in range(NBLK):
                negmx = stats.tile([PB, 1], F32, tag="negmx")
                nc.vector.reduce_max(negmx[:], pt_sb[:, c, :], axis=AX, negate=True)
                ssum = stats.tile([PB, 1], F32, tag="ssum")
                ex = stats.tile([PB, C], F32, tag="ex")
                nc.scalar.activation(
                    ex[:],
                    pt_sb[:, c, :],
                    ACTF.Exp,
                    bias=negmx[:],
                    accum_out=ssum[:],
                )
                rs = stats.tile([PB, 1], F32, tag="rs")
                nc.vector.reciprocal(rs[:], ssum[:])
                nc.vector.tensor_scalar_mul(pa_sb[:, c, :], ex[:], rs[:])

            for ib in range(NBLK):
                S_ps = psS.tile([PB, TOK], F32)
                for ci, (m0, nh) in enumerate(_chunk_plan(ib)):
                    t = chunk_tiles.pop((ib, ci), None)
                    if t is None:
                        t = big.tile([PB, 4, TOK], F32R, tag="chunk")
                        nc.gpsimd.dma_start(
                            out=t[:, 0:nh, :], in_=_head_src(enc, dec, ib, m0, nh)
                        )
                    for hm in range(nh):
                        m = m0 + hm
                        src = t[:, hm, :]
                        s_m = stats.tile([PB, 1], F32, tag="s_m")
                        if m % 2 == 0:
                            nc.vector.reduce_sum(s_m[:], src.bitcast(F32), axis=AX)
                        else:
                            scr = spool.tile([PB, TOK], F32, tag="scr")
                            nc.scalar.activation(
                                scr[:], src.bitcast(F32), ACTF.Copy, accum_out=s_m[:]
                            )
                        r_m = stats.tile([PB, 1], F32, tag="r_m")
                        nc.vector.reciprocal(r_m[:], s_m[:])
                        dg = stats.tile([PB, PB], F32R, tag="dg")
                        nc.vector.tensor_scalar_mul(dg[:], eye_sb[:], r_m[:])
                        nc.tensor.matmul(
                            S_ps[:, 0:512], dg[:], src[:, 0:512],
                            start=(m == 0), stop=(m == 15),
                        )
                        nc.tensor.matmul(
                            S_ps[:, 512:1024], dg[:], src[:, 512:1024],
                            start=(m == 0), stop=(m == 15),
                        )

                S_sb = spool.tile([PB, TOK], F32, tag="S")
                nc.scalar.copy(out=S_sb[:], in_=S_ps[:])
                T_sb = spool.tile([PB, NBLK, PB], F32R, tag="T")
                for jc in range(NBLK):
                    tp = psT.tile([PB, PB], F32)
                    nc.tensor.transpose(
                        tp[:], S_sb[:, jc * PB : (jc + 1) * PB], eye_sb[:]
                    )
                    nc.scalar.copy(out=T_sb[:, jc, :], in_=tp[:])
                z_ps = psZ.tile([C, PB], F32)
                for jc in range(NBLK):
                    nc.tensor.matmul(
                        z_ps[:], pa_sb[:, jc, :], T_sb[:, jc, :],
                        start=(jc == 0), stop=(jc == NBLK - 1),
                    )
                z_sb = zout.tile([C, PB], F32)
                nc.vector.tensor_copy(z_sb[:], z_ps[:])
                nc.sync.dma_start(out=z[ib, :, :], in_=z_sb[:])

    nc.compile()
    return nc


def _get_nc():
    global _NC
    if _NC is None:
        _NC = _build_nc()
    return _NC


def kernel(preds, low_feats, high_feats, unlabeled_ROIs, targets, attns, decode_attns):
    preds = np.asarray(preds, dtype=np.float32)
    attns = np.asarray(attns, dtype=np.float32)
    decode_attns = np.asarray(decode_attns, dtype=np.float32)
    roi = np.asarray(unlabeled_ROIs)

    bz = preds.shape[0]
    preds_t = np.ascontiguousarray(
        preds.reshape(bz, C, TOK).transpose(0, 2, 1)
    )  # (bz, 1024, 21)
    eye_np = np.eye(PB, dtype=np.float32)

    nc = _get_nc()
    in_maps = []
    for k in range(8):
        l, b = k // 4, k % 4
        in_maps.append(
            {
                "enc": np.ascontiguousarray(attns[l, b]),
                "dec": np.ascontiguousarray(decode_attns[l, b]),
                "pt": preds_t[b],
                "eye": eye_np,
            }
        )
    res = run_bass_kernel_spmd(nc, in_maps, core_ids=list(range(8)))
    # z per core: (NBLK, C, PB) holding Z^T per block -> (1024, 21)
    zs = np.stack(
        [
            res.results[k]["z"].transpose(0, 2, 1).reshape(TOK, C)
            for k in range(8)
        ]
    )

    # combine: affinity_raw_b = (Z_{l=0,b} + Z_{l=1,b}) / 32
    zb = zs.reshape(2, bz, TOK, C).sum(axis=0) / 32.0
    aff = zb / zb.sum(axis=-1, keepdims=True)

    # host softmax (matches jax.nn.softmax in f32)
    e = np.exp(preds_t - preds_t.max(axis=-1, keepdims=True))
    prob = e / e.sum(axis=-1, keepdims=True)  # (bz, 1024, 21)

    roi_f = roi.astype(np.float32).reshape(bz, TOK, 1)
    n_roi = roi_f.sum()
    loss = (roi_f * np.abs(prob - aff)).sum()
    if n_roi > 0:
        loss = loss / n_roi
    return np.asarray(loss, dtype=np.float32)
